# revision 1
# baseline (speedup 1.0000x reference)
"""Trainium2 Bass kernel for nn_CAM (channel-attention module).

Reference computation per sample (b=16 total):
    xf   = x.reshape(c, h*w)               # [512, 4096] fp32
    attn = softmax(xf @ xf.T, axis=-1)     # [512, 512]
    y    = attn @ xf                       # [512, 4096]
    out  = beta * y + x

Sharding: data-parallel over batch b across 8 NeuronCores (2 samples per
core); the scalar beta is replicated (pre-broadcast to [128, 1] host-side).

Per-core kernel (matmuls in bf16, softmax/epilogue in fp32):
  1. DMA x tile [128, 4096] fp32 in, cast to bf16 on ScalarE.
  2. xf^T on the PE (transpose-mode matmul vs a bf16 identity), 128x128
     blocks packed 4-wide into one PSUM bank, then one [128, 512]
     copyback per n-block into xfT[p, j, c] = xf[c, 128j+p].
     (The DMA-transpose engine is avoided on purpose: its ISA struct has a
     single sync-wait slot and Tile's xbar-hang serialization overflows it.)
  3. matmul1: A[c,:] accumulated over 32 K-tiles into PSUM (N=512/bank).
  4. softmax: DVE reduce_max(negate) -> ScalarE Exp(bias=-max) with fused
     accum_out row-sum -> fold beta/s into P (epilogue is then just +x).
  5. P^T on the PE the same way, matmul2 over 8 N-chunks of 512.
  6. epilogue: DVE add (PSUM + x fp32) -> DMA out.
"""

import numpy as np

import concourse.bass as bass
import concourse.bacc as bacc
import concourse.mybir as mybir
import concourse.tile as tile
from concourse.bass import ts
from concourse.bass_utils import run_bass_kernel_spmd
from concourse.masks import make_identity

N_CORES = 8
P = 128

F32 = mybir.dt.float32
BF16 = mybir.dt.bfloat16


def build_program(S=2, C=512, HW=4096, n_cores=N_CORES):
    """Build the SPMD Bass program for one core holding S samples."""
    CT = C // P        # c-tiles (partition tiles of the channel dim)
    NT = HW // P       # n-blocks (contraction tiles for matmul1)
    NCHUNK = 512       # free-dim chunk for matmul2 / epilogue (one PSUM bank)
    NCH = HW // NCHUNK

    nc = bacc.Bacc(
        "TRN2", target_bir_lowering=False, debug=False, num_devices=n_cores
    )
    x_in = nc.dram_tensor("x", [S, C, HW], F32, kind="ExternalInput").ap()
    beta_in = nc.dram_tensor("beta", [P, 1], F32, kind="ExternalInput").ap()
    out_d = nc.dram_tensor("out", [S, C, HW], F32, kind="ExternalOutput").ap()

    with tile.TileContext(nc) as tc:
        with (
            tc.tile_pool(name="consts", bufs=1) as consts,
            tc.tile_pool(name="xf32", bufs=CT) as xf32_pool,
            tc.tile_pool(name="xbf", bufs=2) as xbf_pool,
            tc.tile_pool(name="xfT", bufs=1) as xfT_pool,
            tc.tile_pool(name="pmat", bufs=2) as p_pool,
            tc.tile_pool(name="ptr", bufs=2) as pt_pool,
            tc.tile_pool(name="stats", bufs=6) as stats_pool,
            tc.tile_pool(name="outsb", bufs=6) as out_pool,
            tc.tile_pool(name="psumA", bufs=2, space="PSUM") as psumA_pool,
            tc.tile_pool(name="psumY", bufs=3, space="PSUM") as psumY_pool,
            tc.tile_pool(name="psumT", bufs=2, space="PSUM") as psumT_pool,
        ):
            beta_bc = consts.tile([P, 1], F32)
            nc.sync.dma_start(beta_bc[:], beta_in)
            ident = consts.tile([P, P], BF16)
            make_identity(nc, ident[:])

            for s in range(S):
                # ---- load fp32, cast to bf16 ----
                x_sb = []
                xb = xbf_pool.tile([P, CT, HW], BF16, tag="xbf")
                for i in range(CT):
                    xt = xf32_pool.tile([P, HW], F32, tag="xf32")
                    nc.sync.dma_start(xt[:], x_in[s, ts(i, P), :])
                    nc.scalar.copy(xb[:, i, :], xt[:])
                    x_sb.append(xt)

                # ---- xf^T on PE: xfT[p, j, c] = xf[c, 128j + p] ----
                xfT = xfT_pool.tile([P, NT, C], BF16, tag="xfT")
                for j in range(NT):
                    tp = psumT_pool.tile([P, C], BF16, tag="psumT")
                    for i in range(CT):
                        nc.tensor.transpose(
                            tp[:, ts(i, P)], xb[:, i, ts(j, P)], ident[:]
                        )
                    nc.scalar.copy(xfT[:, j, :], tp[:])

                # ---- matmul1 (A = xf @ xf^T) + softmax, per c-tile ----
                pm = p_pool.tile([P, CT, C], BF16, tag="pmat")
                for i in range(CT):
                    pa = psumA_pool.tile([P, C], F32, tag="psumA")
                    for j in range(NT):
                        nc.tensor.matmul(
                            pa[:],
                            lhsT=xfT[:, j, ts(i, P)],
                            rhs=xfT[:, j, :],
                            start=(j == 0),
                            stop=(j == NT - 1),
                        )
                    negm = stats_pool.tile([P, 1], F32, tag="negm")
                    nc.vector.reduce_max(
                        negm[:], pa[:], axis=mybir.AxisListType.X, negate=True
                    )
                    ssum = stats_pool.tile([P, 1], F32, tag="ssum")
                    nc.scalar.activation(
                        pm[:, i, :],
                        pa[:],
                        mybir.ActivationFunctionType.Exp,
                        bias=negm[:],
                        scale=1.0,
                        accum_out=ssum[:],
                    )
                    # rb = beta / rowsum; fold into P so epilogue is just +x
                    rinv = stats_pool.tile([P, 1], F32, tag="rinv")
                    nc.vector.reciprocal(rinv[:], ssum[:])
                    rb = stats_pool.tile([P, 1], F32, tag="rb")
                    nc.vector.tensor_scalar_mul(rb[:], rinv[:], beta_bc[:, 0:1])
                    nc.vector.tensor_scalar_mul(pm[:, i, :], pm[:, i, :], rb[:, 0:1])

                # ---- P^T on PE: PT[p, k, c] = (beta*softmax(A))[c, 128k+p] ----
                PT = pt_pool.tile([P, CT, C], BF16, tag="PT")
                for k in range(CT):
                    tp = psumT_pool.tile([P, C], BF16, tag="psumT")
                    for i in range(CT):
                        nc.tensor.transpose(
                            tp[:, ts(i, P)], pm[:, i, ts(k, P)], ident[:]
                        )
                    nc.scalar.copy(PT[:, k, :], tp[:])

                # ---- matmul2 (y = S @ xf) + epilogue (+x), per c-tile ----
                for i in range(CT):
                    for n in range(NCH):
                        py = psumY_pool.tile([P, NCHUNK], F32, tag="psumY")
                        for k in range(CT):
                            nc.tensor.matmul(
                                py[:],
                                lhsT=PT[:, k, ts(i, P)],
                                rhs=xb[:, k, ts(n, NCHUNK)],
                                start=(k == 0),
                                stop=(k == CT - 1),
                            )
                        ot = out_pool.tile([P, NCHUNK], F32, tag="outsb")
                        nc.vector.tensor_add(
                            out=ot[:],
                            in0=py[:],
                            in1=x_sb[i][:, ts(n, NCHUNK)],
                        )
                        nc.sync.dma_start(
                            out_d[s, ts(i, P), ts(n, NCHUNK)], ot[:]
                        )

    nc.compile()
    return nc


_PROGRAM_CACHE = {}


def _get_program(S, C, HW, n_cores):
    key = (S, C, HW, n_cores)
    if key not in _PROGRAM_CACHE:
        _PROGRAM_CACHE[key] = build_program(S, C, HW, n_cores)
    return _PROGRAM_CACHE[key]


def kernel(x: np.ndarray, beta: np.ndarray) -> np.ndarray:
    b, c, h, w = x.shape
    assert (b, c, h, w) == (16, 512, 64, 64), f"unexpected shape {x.shape}"
    hw = h * w
    S = b // N_CORES

    nc = _get_program(S, c, hw, N_CORES)

    xf = np.ascontiguousarray(
        np.asarray(x, dtype=np.float32).reshape(b, c, hw)
    )
    beta_bc = np.ascontiguousarray(
        np.broadcast_to(
            np.asarray(beta, dtype=np.float32).reshape(1, 1), (P, 1)
        )
    )

    in_maps = [
        {"x": xf[core * S : (core + 1) * S], "beta": beta_bc}
        for core in range(N_CORES)
    ]
    res = run_bass_kernel_spmd(nc, in_maps, list(range(N_CORES)))

    out = np.empty((b, c, hw), dtype=np.float32)
    for core in range(N_CORES):
        out[core * S : (core + 1) * S] = res.results[core]["out"]
    return out.reshape(b, c, h, w)

